# revision 36
# baseline (speedup 1.0000x reference)
"""PlatonicConv (graph-mode attention) Trainium2 Bass kernel.

Math (per graph of 64 fully-connected nodes, 24 group-heads of dim 16):
  q/k/v = x @ W; RoPE(q, k) from pos; S = q.k^T/4; softmax over dst;
  out = A @ v; y = out @ Wo.  32 graphs -> data-parallel over 8 cores.

v3 design (per core: 4 graphs, 256 nodes), all matmul operands bf16
(full PE rate at any free dim; fp32/f32r are 4x slower below 256 cols):
  * x^T via PE transposes; q/k projected transposed ([feature, node]).
  * RoPE pair-swap folds into spread matrices: rot_spread accumulates
    esp@(q*cos) + espp@(q*sin) in PSUM -- no Wqp/Wkp weights.
  * theta range-reduction: two chained add_range_wrap custom-DVE ops
    (valid for |theta| < 4.5pi; actual max ~10.8), one Sin table.
  * Q/K spreads use a parity-interleaved layout (graph parity g01 at
    16-row offset inside each 32-slot) so ONE block-diagonal matmul
    computes both parities' scores: 48 score matmuls instead of 96.
  * Softmax is max-free (scores O(1) by construction); denominators are
    a 17th row of each AV matmul via an interleaved ones-column in V,
    extracted/broadcast with tiny PE matmuls, reciprocal_approx_fast.
  * AV output stays in spread layout; Wo rows are pre-spread host-side
    (wo_sp); final y lands in natural node order -> dense DRAM writes.
  * DMA queues: sync HWDGE = x/wq/wvil/y0; scalar HWDGE = small consts
    + y1 (keeps the Sin/Exp activations unblocked); gpsimd SWDGE =
    everything else (wk, wo_sp, spread/selector matrices).
"""

import numpy as np
import ml_dtypes

G = 12
H = 2
D = 16
GH = 24          # G * H group-heads
C = 384          # in/emb/out channels
NG = 32          # graphs
NPG = 64         # nodes per graph
N = NG * NPG
NCORES = 8
GPC = NG // NCORES   # graphs per core = 4
NPC = GPC * NPG      # nodes per core = 256
VW = 17              # V block width (16 + ones col)
CAUG = GH * VW       # 408

BF = ml_dtypes.bfloat16

_CACHE = {}


def _host_prep(Wq, Wk, Wv, Wo, rope_freqs):
    f32 = np.float32

    def pack(w):
        # [384, cols] -> [128, 3*cols]: row p = concat_s w[128 s + p]
        cols = w.shape[1]
        return np.ascontiguousarray(
            w.reshape(3, 128, cols).transpose(1, 0, 2).reshape(128, 3 * cols)
            .astype(BF))

    # V interleaved with a ones column per head: block j = [Wv head j | 0]
    Wvil = np.zeros((C, CAUG), f32)
    for j in range(GH):
        Wvil[:, VW * j:VW * j + 16] = Wv[:, 16 * j:16 * j + 16]
    vseed = np.zeros((1, CAUG), f32)
    vseed[0, VW * np.arange(GH) + 16] = 1.0

    # theta pattern [3, 128] for COMPACT layout rows (dup to both halves):
    # row r=16m+d of a 64-block -> head h=m%2, pair w=d//2
    fr = rope_freqs.astype(f32)            # [3, 2, 8]
    fpat = np.zeros((3, 128), f32)
    for r in range(128):
        rr = r % 64
        fpat[:, r] = fr[:, (rr // 16) % 2, (rr % 16) // 2]

    # spread matrices, parity-interleaved: compact row k (16/head) of a
    # 64-row half -> slot row 32s + 16*P + j for graph parity P. Variant
    # 2P   = plain (multiplies q*cos), variant 2P+1 = pair-swap signed
    # (multiplies q*sin). Two stacked copies for odd 64-row slabs.
    espx = np.zeros((128, 4, 128), f32)
    for k in range(64):
        s, j = divmod(k, 16)
        jp = j + 1 if j % 2 == 0 else j - 1
        sg = 1.0 if j % 2 == 0 else -1.0
        for P in range(2):
            espx[k, 2 * P, 32 * s + 16 * P + j] = 1.0
            espx[64 + k, 2 * P, 32 * s + 16 * P + j] = 1.0
            espx[k, 2 * P + 1, 32 * s + 16 * P + jp] = sg
            espx[64 + k, 2 * P + 1, 32 * s + 16 * P + jp] = sg

    # den extract: per quad qd, spread row 32a+16 -> out row 4qd+a of a
    # [24,128] PSUM accumulation (out base partition must be 32-aligned,
    # so each quad writes the full 24 rows, 4 nonzero)
    esel = np.zeros((128, 6 * GH), f32)
    for qd in range(6):
        for a in range(4):
            esel[32 * a + 16, GH * qd + 4 * qd + a] = 1.0

    # den broadcast: rden row gh -> 16 spread rows of its quad block
    bsel = np.zeros((GH, 6 * 128), f32)
    for gh in range(GH):
        qd, a = divmod(gh, 4)
        bsel[gh, 128 * qd + 32 * a:128 * qd + 32 * a + 16] = 1.0

    # Wo with rows pre-spread to the AV output layout (den/pad rows = 0)
    wo_sp = np.zeros((128, 6 * C), f32)
    for gh in range(GH):
        qd, a = divmod(gh, 4)
        wo_sp[32 * a:32 * a + 16, C * qd:C * qd + C] = Wo[16 * gh:16 * gh + 16]

    return dict(
        wq=pack(Wq), wk=pack(Wk), wvil=pack(Wvil),
        wo_sp=wo_sp.astype(BF), vseed=vseed.astype(BF),
        fpat=fpat,
        espx=np.ascontiguousarray(espx.reshape(128, 4 * 128)).astype(BF),
        esel=esel.astype(BF), bsel=bsel.astype(BF),
        onesrow=np.ones((1, 128), BF), ident=np.eye(128, dtype=BF),
    )


def _build_nc():
    import concourse.bacc as bacc
    import concourse.tile as tile
    import concourse.mybir as mybir
    from contextlib import ExitStack

    f32 = mybir.dt.float32
    bf16 = mybir.dt.bfloat16
    AF = mybir.ActivationFunctionType

    nc = bacc.Bacc("TRN2", target_bir_lowering=False)

    x_d = nc.dram_tensor("x", [128, 2 * C], bf16, kind="ExternalInput")
    posT_d = nc.dram_tensor("posT", [3, NPC], f32, kind="ExternalInput")
    wq_d = nc.dram_tensor("wq", [128, 3 * C], bf16, kind="ExternalInput")
    wk_d = nc.dram_tensor("wk", [128, 3 * C], bf16, kind="ExternalInput")
    wvil_d = nc.dram_tensor("wvil", [128, 3 * CAUG], bf16, kind="ExternalInput")
    wo_sp_d = nc.dram_tensor("wo_sp", [128, 6 * C], bf16, kind="ExternalInput")
    vseed_d = nc.dram_tensor("vseed", [1, CAUG], bf16, kind="ExternalInput")
    fpat_d = nc.dram_tensor("fpat", [3, 128], f32, kind="ExternalInput")
    espx_d = nc.dram_tensor("espx", [128, 4 * 128], bf16, kind="ExternalInput")
    esel_d = nc.dram_tensor("esel", [128, 6 * GH], bf16, kind="ExternalInput")
    bsel_d = nc.dram_tensor("bsel", [GH, 6 * 128], bf16, kind="ExternalInput")
    ones_d = nc.dram_tensor("onesrow", [1, 128], bf16, kind="ExternalInput")
    ident_d = nc.dram_tensor("ident", [128, 128], bf16, kind="ExternalInput")
    y_d = nc.dram_tensor("y", [NPC, C], f32, kind="ExternalOutput")

    ctx = ExitStack()
    with tile.TileContext(nc) as tc, ctx:
        consts = ctx.enter_context(tc.tile_pool(name="consts", bufs=1))
        wpool = ctx.enter_context(tc.tile_pool(name="weights", bufs=1))
        sb = ctx.enter_context(tc.tile_pool(name="sbuf", bufs=1))
        # 2+2+4 PSUM banks: gp (proj/theta/vau/den/bcast/y), sp (spread/AV),
        # att (score tiles; bank gh%4 for 4-way concurrent row groups)
        ps_gp = ctx.enter_context(tc.tile_pool(name="ps_gp", bufs=2, space="PSUM"))
        ps_sp = ctx.enter_context(tc.tile_pool(name="ps_sp", bufs=2, space="PSUM"))
        ps_att = ctx.enter_context(tc.tile_pool(name="ps_att", bufs=1, space="PSUM"))

        def gpt(shape, dt=f32):
            return ps_gp.tile(shape, dt, tag="pp", name="pp")

        def spt(shape, dt=f32):
            return ps_sp.tile(shape, dt, tag="sp", name="sp")

        # ---- inputs; x/wq/wvil on sync HWDGE, tiny consts on scalar HWDGE,
        # the rest on gpsimd SWDGE so the scalar engine stays free ----
        ident = consts.tile([128, 128], bf16, tag="ident")
        nc.sync.dma_start(out=ident, in_=ident_d[:])
        xsb = sb.tile([128, 2, C], bf16, tag="x")
        nc.sync.dma_start(out=xsb, in_=x_d.rearrange("p (s e) -> p s e", s=2))
        posT = consts.tile([3, NPC], f32, tag="posT")
        nc.sync.dma_start(out=posT, in_=posT_d[:])
        fpat = consts.tile([3, 128], f32, tag="fpat")
        nc.sync.dma_start(out=fpat, in_=fpat_d[:])
        espx = consts.tile([128, 4, 128], bf16, tag="espx")
        nc.gpsimd.dma_start(out=espx, in_=espx_d.rearrange("p (v e) -> p v e", v=4))
        esel = consts.tile([128, 6 * GH], bf16, tag="esel")
        nc.gpsimd.dma_start(out=esel, in_=esel_d[:])
        bsel = consts.tile([GH, 6 * 128], bf16, tag="bsel")
        nc.gpsimd.dma_start(out=bsel, in_=bsel_d[:])
        vseed = consts.tile([1, CAUG], bf16, tag="vseed")
        nc.gpsimd.dma_start(out=vseed, in_=vseed_d[:])
        onesrow = consts.tile([1, 128], bf16, tag="ones")
        nc.gpsimd.dma_start(out=onesrow, in_=ones_d[:])

        wq = wpool.tile([128, 3, C], bf16, tag="wq")
        nc.sync.dma_start(out=wq, in_=wq_d.rearrange("p (s e) -> p s e", s=3))
        wvil = wpool.tile([128, 3, CAUG], bf16, tag="wvil")
        nc.sync.dma_start(out=wvil, in_=wvil_d.rearrange("p (s e) -> p s e", s=3))
        wk = wpool.tile([128, 3, C], bf16, tag="wk")
        nc.gpsimd.dma_start(out=wk, in_=wk_d.rearrange("p (s e) -> p s e", s=3))
        wo_sp = wpool.tile([128, 6 * C], bf16, tag="wo_sp")
        nc.gpsimd.dma_start(out=wo_sp, in_=wo_sp_d[:])

        # ---- PE warmup: the HAM clock gate holds the array at 1.2 GHz
        # until ~3.4us of sustained activity.  The PE would otherwise idle
        # 7.5-11us waiting on weight DMAs, so burn that window on dummy
        # matmuls (ident arrives first on sync); results are never read.
        # Uses the ps_att slot, untouched until the first scores (~20us). ----
        warm = ps_att.tile([128, 128], f32, tag="stps", name="warm")
        for _ in range(24):
            nc.tensor.matmul(out=warm, lhsT=ident, rhs=ident,
                             start=True, stop=True)

        # ---- X^T [384, 256] via PE transposes (bf16) ----
        xT = []
        for j in range(3):
            t = sb.tile([128, NPC], bf16, tag=f"xT{j}")
            for i in range(2):
                pst = gpt([128, 128], bf16)
                nc.tensor.transpose(
                    out=pst, in_=xsb[:, i, 128 * j:128 * j + 128], identity=ident)
                nc.vector.tensor_copy(out=t[:, 128 * i:128 * i + 128], in_=pst)
            xT.append(t)

        # ---- theta [128, 256]; range-reduce via 2 chained add_range_wrap
        # (safe for |theta| < 4.5pi); Sin table gives both sin and cos ----
        PI = float(np.pi)
        thps = gpt([128, NPC])
        nc.tensor.matmul(
            out=thps, lhsT=fpat,
            rhs=posT,
            start=True, stop=True)
        cpat = sb.tile([128, NPC], f32, tag="cpat")
        spat = sb.tile([128, NPC], f32, tag="spat")
        for (tgt, shift) in ((spat, 0.0), (cpat, PI / 2)):
            w1 = sb.tile([128, NPC], f32, tag=f"w1{shift}", name="w1")
            nc.vector.add_range_wrap(out=w1, in_=thps, shift=shift,
                                     bound=PI, period=2 * PI)
            w2 = sb.tile([128, NPC], f32, tag=f"w2{shift}", name="w2")
            nc.vector.add_range_wrap(out=w2, in_=w1, shift=0.0,
                                     bound=PI, period=2 * PI)
            nc.scalar.activation(out=tgt, in_=w2, func=AF.Sin)

        # ---- projections (transposed) + RoPE + parity-interleaved spread.
        # qz[t] [128,128]: slot rows 32s+16P+j, cols (pair, src i) -- both
        # parities' q stacked per column.  kz[t] [128, (pr, P, i)]: same rows,
        # block-diag in data (parity P data only in P's 16-row sub-slot). ----
        def proj_m(w, m):
            ps = gpt([128, NPC])
            for k in range(3):
                nc.tensor.matmul(
                    out=ps,
                    lhsT=w[:, k, 128 * m:128 * m + 128],
                    rhs=xT[k],
                    start=(k == 0), stop=(k == 2))
            return ps

        qz, kz = [], []

        def copy_scalar(out, in_):
            # psum->sbuf copy on the scalar engine (idle during proj phase)
            nc.scalar.activation(out=out, in_=in_, func=AF.Copy)

        for m in range(3):
            for (w, out_tiles, isq, tg) in ((wq, qz, True, "q"), (wk, kz, False, "k")):
                qt = proj_m(w, m)
                a = sb.tile([128, 2, 2, 64], bf16, tag=f"ra{tg}{m}")
                b = sb.tile([128, 2, 2, 64], bf16, tag=f"rb{tg}{m}")
                nc.vector.tensor_mul(out=a, in0=qt, in1=cpat)
                nc.vector.tensor_mul(out=b, in0=qt, in1=spat)
                for half in range(2):
                    hs = slice(64 * half, 64 * half + 64)
                    if isq:
                        sp = spt([128, 128])
                        for P in range(2):
                            nc.tensor.matmul(
                                out=sp, lhsT=espx[hs, 2 * P, :],
                                rhs=a[hs, :, P, :],
                                start=(P == 0), stop=False)
                            nc.tensor.matmul(
                                out=sp, lhsT=espx[hs, 2 * P + 1, :],
                                rhs=b[hs, :, P, :],
                                start=False, stop=(P == 1))
                        t = sb.tile([128, 128], bf16, tag=f"sps{tg}{2 * m + half}")
                    else:
                        sp = spt([128, 2, 2, 64])
                        for P in range(2):
                            nc.tensor.matmul(
                                out=sp[:, :, P, :], lhsT=espx[hs, 2 * P, :],
                                rhs=a[hs, :, P, :],
                                start=True, stop=False)
                            nc.tensor.matmul(
                                out=sp[:, :, P, :], lhsT=espx[hs, 2 * P + 1, :],
                                rhs=b[hs, :, P, :],
                                start=False, stop=True)
                        t = sb.tile([128, 2, 2, 64], bf16,
                                    tag=f"sps{tg}{2 * m + half}")
                    nc.vector.tensor_copy(out=t, in_=sp)
                    out_tiles.append(t)

        # ---- V_aug [256, 408] untransposed (+ ones cols via K=1 matmul) ----
        vau = []
        for i in range(2):
            ps = gpt([128, CAUG])
            for k in range(3):
                nc.tensor.matmul(
                    out=ps,
                    lhsT=xT[k][:, 128 * i:128 * i + 128],
                    rhs=wvil[:, k, :],
                    start=(k == 0), stop=False)
            nc.tensor.matmul(
                out=ps, lhsT=onesrow, rhs=vseed,
                start=False, stop=True)
            t = sb.tile([128, CAUG], bf16, tag=f"vau{i}")
            copy_scalar(t, ps)
            vau.append(t)

        # ---- scores S^T + exp, per graph-pair: ONE block-diag matmul per
        # (gh, pair) covers both parities.  head gh -> bank gh%4 (512-col
        # block), col 64*(gh//4); rows 64*P via the kz data layout. ----
        def scol(gh):
            return 512 * (gh % 4) + 64 * (gh // 4)

        expst = []
        for pair in range(2):
            stp = ps_att.tile([128, 4 * 512], f32, tag="stps")
            et = sb.tile([128, 4 * 512], bf16, tag=f"expst{pair}")
            # 2 chunks per bank; chunk 0 (tiles 0-2 cols) is emitted as
            # soon as its scores are done so AV quads 0-2 start early
            for gh in range(GH):
                tilei, slot = divmod(gh, 4)
                lo = 32 * slot
                nc.tensor.matmul(
                    out=stp[:, scol(gh):scol(gh) + 64],
                    lhsT=kz[tilei][lo:lo + 32, pair, :, :],
                    rhs=qz[tilei][lo:lo + 32, 64 * pair:64 * pair + 64],
                    start=True, stop=True,
                    tile_position=(lo, 0))
                if gh == 11:
                    for b in range(4):
                        co = 512 * b
                        nc.scalar.activation(
                            out=et[:, co:co + 192], in_=stp[:, co:co + 192],
                            func=AF.Exp, scale=0.25)
            for b in range(4):
                co = 512 * b + 192
                nc.scalar.activation(
                    out=et[:, co:co + 192], in_=stp[:, co:co + 192],
                    func=AF.Exp, scale=0.25)
            expst.append(et)

        # ---- AV (+den row) per (pair, quad): two [128,64] psum tiles so
        # concurrent graph parities use distinct banks. avsb columns land in
        # natural node order: col = 256*qd + 128*pair + 64*g01 + i64.
        # (no memsets: psum slots hold finite data from the spread phase,
        # and pad rows are zero-masked by rdsp/wo_sp downstream) ----
        avsb = sb.tile([128, 6 * 256], bf16, tag="avsb")
        for pair in range(2):
            for qd in range(6):
                mk = spt if (qd % 2 == 0) else (lambda sh: gpt(sh))
                avt = [mk([128, 64]) for _ in range(2)]
                for a in range(4):
                    gh = 4 * qd + a
                    for g01 in range(2):
                        nc.tensor.matmul(
                            out=avt[g01][32 * a:32 * a + VW, :],
                            lhsT=vau[pair][64 * g01:64 * g01 + 64,
                                           VW * gh:VW * gh + VW],
                            rhs=expst[pair][64 * g01:64 * g01 + 64,
                                            scol(gh):scol(gh) + 64],
                            start=True, stop=True,
                            tile_position=(64 * g01, 32 * a))
                cbase = 256 * qd + 128 * pair
                for g01 in range(2):
                    nc.vector.tensor_copy(
                        out=avsb[:, cbase + 64 * g01:cbase + 64 * g01 + 64],
                        in_=avt[g01])

        # ---- merged tail: den extract (both pairs per quad), reciprocal,
        # broadcast, normalize, project.  den rows (32a+16) -> denps rows
        # 4qd+a (full [24,256] write with 4 nonzero rows, accumulated) ----
        denps = gpt([GH, 2 * 128])
        for qd in range(6):
            nc.tensor.matmul(
                out=denps,
                lhsT=esel[:, GH * qd:GH * qd + GH],
                rhs=avsb[:, 256 * qd:256 * qd + 256],
                start=(qd == 0), stop=(qd == 5))
        rden = sb.tile([GH, 2 * 128], f32, tag="rden")
        nc.vector.reciprocal_approx_fast(out=rden, in_=denps)
        rdenb = sb.tile([GH, 2 * 128], bf16, tag="rdenb")
        nc.vector.tensor_copy(out=rdenb, in_=rden)
        yps = [gpt([128, C]) for _ in range(2)]
        for qd in range(6):
            rdsp = spt([128, 2 * 128])
            nc.tensor.matmul(
                out=rdsp, lhsT=bsel[:, 128 * qd:128 * qd + 128],
                rhs=rdenb, start=True, stop=True)
            ot = sb.tile([128, 2 * 128], bf16, tag=f"onrm{qd}", name="onrm")
            nc.vector.tensor_mul(
                out=ot, in0=avsb[:, 256 * qd:256 * qd + 256], in1=rdsp)
            for pair in range(2):
                nc.tensor.matmul(
                    out=yps[pair], lhsT=ot[:, 128 * pair:128 * pair + 128],
                    rhs=wo_sp[:, C * qd:C * qd + C],
                    start=(qd == 0), stop=(qd == 5))
        for pair in range(2):
            yt = sb.tile([128, C], f32, tag=f"ysb{pair}", name="ysb")
            nc.vector.tensor_copy(out=yt, in_=yps[pair])
            (nc.sync if pair == 0 else nc.scalar).dma_start(
                out=y_d[128 * pair:128 * pair + 128, :], in_=yt)

    nc.compile()
    return nc


def _get_nc():
    if "nc" not in _CACHE:
        _CACHE["nc"] = _build_nc()
    return _CACHE["nc"]


def make_in_maps(inputs):
    x = np.asarray(inputs["x"], np.float32)
    pos = np.asarray(inputs["pos"], np.float32)
    prep = _host_prep(np.asarray(inputs["Wq"], np.float32),
                      np.asarray(inputs["Wk"], np.float32),
                      np.asarray(inputs["Wv"], np.float32),
                      np.asarray(inputs["Wo"], np.float32),
                      np.asarray(inputs["rope_freqs"], np.float32))
    in_maps = []
    for c in range(NCORES):
        sl = slice(c * NPC, (c + 1) * NPC)
        m = dict(prep)
        xs = x[sl]
        m["x"] = np.ascontiguousarray(
            xs.reshape(2, 128, C).transpose(1, 0, 2).reshape(128, 2 * C)
            .astype(BF))
        m["posT"] = np.ascontiguousarray(pos[sl].T.astype(np.float32))
        in_maps.append(m)
    return in_maps


def kernel(**inputs):
    from concourse.bass_utils import run_bass_kernel_spmd

    in_maps = make_in_maps(inputs)

    nc = _get_nc()
    res = run_bass_kernel_spmd(nc, in_maps, core_ids=list(range(NCORES)))
    out = np.concatenate([res.results[c]["y"] for c in range(NCORES)], axis=0)
    return out.astype(np.float32)


# revision 37
# speedup vs baseline: 1.0170x; 1.0170x over previous
"""PlatonicConv (graph-mode attention) Trainium2 Bass kernel.

Math (per graph of 64 fully-connected nodes, 24 group-heads of dim 16):
  q/k/v = x @ W; RoPE(q, k) from pos; S = q.k^T/4; softmax over dst;
  out = A @ v; y = out @ Wo.  32 graphs -> data-parallel over 8 cores.

v3 design (per core: 4 graphs, 256 nodes), all matmul operands bf16
(full PE rate at any free dim; fp32/f32r are 4x slower below 256 cols):
  * x^T via PE transposes; q/k projected transposed ([feature, node]).
  * RoPE pair-swap folds into spread matrices: rot_spread accumulates
    esp@(q*cos) + espp@(q*sin) in PSUM -- no Wqp/Wkp weights.
  * theta range-reduction: two chained add_range_wrap custom-DVE ops
    (valid for |theta| < 4.5pi; actual max ~10.8), one Sin table.
  * Q/K spreads use a parity-interleaved layout (graph parity g01 at
    16-row offset inside each 32-slot) so ONE block-diagonal matmul
    computes both parities' scores: 48 score matmuls instead of 96.
  * Softmax is max-free (scores O(1) by construction); denominators are
    a 17th row of each AV matmul via an interleaved ones-column in V,
    extracted/broadcast with tiny PE matmuls, reciprocal_approx_fast.
  * AV output stays in spread layout; Wo rows are pre-spread host-side
    (wo_sp); final y lands in natural node order -> dense DRAM writes.
  * DMA queues: sync HWDGE = x/wq/wvil/y0; scalar HWDGE = small consts
    + y1 (keeps the Sin/Exp activations unblocked); gpsimd SWDGE =
    everything else (wk, wo_sp, spread/selector matrices).
"""

import numpy as np
import ml_dtypes

G = 12
H = 2
D = 16
GH = 24          # G * H group-heads
C = 384          # in/emb/out channels
NG = 32          # graphs
NPG = 64         # nodes per graph
N = NG * NPG
NCORES = 8
GPC = NG // NCORES   # graphs per core = 4
NPC = GPC * NPG      # nodes per core = 256
VW = 17              # V block width (16 + ones col)
CAUG = GH * VW       # 408

BF = ml_dtypes.bfloat16

_CACHE = {}


def _host_prep(Wq, Wk, Wv, Wo, rope_freqs):
    f32 = np.float32

    def pack(w):
        # [384, cols] -> [128, 3*cols]: row p = concat_s w[128 s + p]
        cols = w.shape[1]
        return np.ascontiguousarray(
            w.reshape(3, 128, cols).transpose(1, 0, 2).reshape(128, 3 * cols)
            .astype(BF))

    # V interleaved with a ones column per head: block j = [Wv head j | 0]
    Wvil = np.zeros((C, CAUG), f32)
    for j in range(GH):
        Wvil[:, VW * j:VW * j + 16] = Wv[:, 16 * j:16 * j + 16]
    vseed = np.zeros((1, CAUG), f32)
    vseed[0, VW * np.arange(GH) + 16] = 1.0

    # theta pattern [3, 128] for COMPACT layout rows (dup to both halves):
    # row r=16m+d of a 64-block -> head h=m%2, pair w=d//2
    fr = rope_freqs.astype(f32)            # [3, 2, 8]
    fpat = np.zeros((3, 128), f32)
    for r in range(128):
        rr = r % 64
        fpat[:, r] = fr[:, (rr // 16) % 2, (rr % 16) // 2]

    # spread matrices, parity-interleaved: compact row k (16/head) of a
    # 64-row half -> slot row 32s + 16*P + j for graph parity P. Variant
    # 2P   = plain (multiplies q*cos), variant 2P+1 = pair-swap signed
    # (multiplies q*sin). Two stacked copies for odd 64-row slabs.
    espx = np.zeros((128, 4, 128), f32)
    for k in range(64):
        s, j = divmod(k, 16)
        jp = j + 1 if j % 2 == 0 else j - 1
        sg = 1.0 if j % 2 == 0 else -1.0
        for P in range(2):
            espx[k, 2 * P, 32 * s + 16 * P + j] = 1.0
            espx[64 + k, 2 * P, 32 * s + 16 * P + j] = 1.0
            espx[k, 2 * P + 1, 32 * s + 16 * P + jp] = sg
            espx[64 + k, 2 * P + 1, 32 * s + 16 * P + jp] = sg

    # den extract: per quad qd, spread row 32a+16 -> out row 4qd+a of a
    # [24,128] PSUM accumulation (out base partition must be 32-aligned,
    # so each quad writes the full 24 rows, 4 nonzero)
    esel = np.zeros((128, 6 * GH), f32)
    for qd in range(6):
        for a in range(4):
            esel[32 * a + 16, GH * qd + 4 * qd + a] = 1.0

    # den broadcast: rden row gh -> 16 spread rows of its quad block
    bsel = np.zeros((GH, 6 * 128), f32)
    for gh in range(GH):
        qd, a = divmod(gh, 4)
        bsel[gh, 128 * qd + 32 * a:128 * qd + 32 * a + 16] = 1.0

    # Wo with rows pre-spread to the AV output layout (den/pad rows = 0)
    wo_sp = np.zeros((128, 6 * C), f32)
    for gh in range(GH):
        qd, a = divmod(gh, 4)
        wo_sp[32 * a:32 * a + 16, C * qd:C * qd + C] = Wo[16 * gh:16 * gh + 16]

    return dict(
        wq=pack(Wq), wk=pack(Wk), wvil=pack(Wvil),
        wo_sp=wo_sp.astype(BF), vseed=vseed.astype(BF),
        fpat=fpat,
        espx=np.ascontiguousarray(espx.reshape(128, 4 * 128)).astype(BF),
        esel=esel.astype(BF), bsel=bsel.astype(BF),
        onesrow=np.ones((1, 128), BF), ident=np.eye(128, dtype=BF),
    )


def _build_nc():
    import concourse.bacc as bacc
    import concourse.tile as tile
    import concourse.mybir as mybir
    from contextlib import ExitStack

    f32 = mybir.dt.float32
    bf16 = mybir.dt.bfloat16
    AF = mybir.ActivationFunctionType

    nc = bacc.Bacc("TRN2", target_bir_lowering=False)

    x_d = nc.dram_tensor("x", [128, 2 * C], bf16, kind="ExternalInput")
    posT_d = nc.dram_tensor("posT", [3, NPC], f32, kind="ExternalInput")
    wq_d = nc.dram_tensor("wq", [128, 3 * C], bf16, kind="ExternalInput")
    wk_d = nc.dram_tensor("wk", [128, 3 * C], bf16, kind="ExternalInput")
    wvil_d = nc.dram_tensor("wvil", [128, 3 * CAUG], bf16, kind="ExternalInput")
    wo_sp_d = nc.dram_tensor("wo_sp", [128, 6 * C], bf16, kind="ExternalInput")
    vseed_d = nc.dram_tensor("vseed", [1, CAUG], bf16, kind="ExternalInput")
    fpat_d = nc.dram_tensor("fpat", [3, 128], f32, kind="ExternalInput")
    espx_d = nc.dram_tensor("espx", [128, 4 * 128], bf16, kind="ExternalInput")
    esel_d = nc.dram_tensor("esel", [128, 6 * GH], bf16, kind="ExternalInput")
    bsel_d = nc.dram_tensor("bsel", [GH, 6 * 128], bf16, kind="ExternalInput")
    ones_d = nc.dram_tensor("onesrow", [1, 128], bf16, kind="ExternalInput")
    ident_d = nc.dram_tensor("ident", [128, 128], bf16, kind="ExternalInput")
    y_d = nc.dram_tensor("y", [NPC, C], f32, kind="ExternalOutput")

    ctx = ExitStack()
    with tile.TileContext(nc) as tc, ctx:
        consts = ctx.enter_context(tc.tile_pool(name="consts", bufs=1))
        wpool = ctx.enter_context(tc.tile_pool(name="weights", bufs=1))
        sb = ctx.enter_context(tc.tile_pool(name="sbuf", bufs=1))
        # 2+2+4 PSUM banks: gp (proj/theta/vau/den/bcast/y), sp (spread/AV),
        # att (score tiles; bank gh%4 for 4-way concurrent row groups)
        ps_gp = ctx.enter_context(tc.tile_pool(name="ps_gp", bufs=2, space="PSUM"))
        ps_sp = ctx.enter_context(tc.tile_pool(name="ps_sp", bufs=2, space="PSUM"))
        ps_att = ctx.enter_context(tc.tile_pool(name="ps_att", bufs=1, space="PSUM"))

        def gpt(shape, dt=f32):
            return ps_gp.tile(shape, dt, tag="pp", name="pp")

        def spt(shape, dt=f32):
            return ps_sp.tile(shape, dt, tag="sp", name="sp")

        # ---- inputs; x/wq/wvil on sync HWDGE, tiny consts on scalar HWDGE,
        # the rest on gpsimd SWDGE so the scalar engine stays free ----
        ident = consts.tile([128, 128], bf16, tag="ident")
        nc.sync.dma_start(out=ident, in_=ident_d[:])
        xsb = sb.tile([128, 2, C], bf16, tag="x")
        nc.sync.dma_start(out=xsb, in_=x_d.rearrange("p (s e) -> p s e", s=2))
        posT = consts.tile([3, NPC], f32, tag="posT")
        nc.sync.dma_start(out=posT, in_=posT_d[:])
        fpat = consts.tile([3, 128], f32, tag="fpat")
        nc.sync.dma_start(out=fpat, in_=fpat_d[:])
        espx = consts.tile([128, 4, 128], bf16, tag="espx")
        nc.gpsimd.dma_start(out=espx, in_=espx_d.rearrange("p (v e) -> p v e", v=4))
        esel = consts.tile([128, 6 * GH], bf16, tag="esel")
        nc.gpsimd.dma_start(out=esel, in_=esel_d[:])
        bsel = consts.tile([GH, 6 * 128], bf16, tag="bsel")
        nc.gpsimd.dma_start(out=bsel, in_=bsel_d[:])
        vseed = consts.tile([1, CAUG], bf16, tag="vseed")
        nc.gpsimd.dma_start(out=vseed, in_=vseed_d[:])
        onesrow = consts.tile([1, 128], bf16, tag="ones")
        nc.gpsimd.dma_start(out=onesrow, in_=ones_d[:])

        wq = wpool.tile([128, 3, C], bf16, tag="wq")
        nc.sync.dma_start(out=wq, in_=wq_d.rearrange("p (s e) -> p s e", s=3))
        wvil = wpool.tile([128, 3, CAUG], bf16, tag="wvil")
        nc.sync.dma_start(out=wvil, in_=wvil_d.rearrange("p (s e) -> p s e", s=3))
        wk = wpool.tile([128, 3, C], bf16, tag="wk")
        nc.gpsimd.dma_start(out=wk, in_=wk_d.rearrange("p (s e) -> p s e", s=3))
        wo_sp = wpool.tile([128, 6 * C], bf16, tag="wo_sp")
        nc.gpsimd.dma_start(out=wo_sp, in_=wo_sp_d[:])

        # ---- X^T [384, 256] via PE transposes (bf16) ----
        xT = []
        for j in range(3):
            t = sb.tile([128, NPC], bf16, tag=f"xT{j}")
            for i in range(2):
                pst = gpt([128, 128], bf16)
                nc.tensor.transpose(
                    out=pst, in_=xsb[:, i, 128 * j:128 * j + 128], identity=ident)
                nc.vector.tensor_copy(out=t[:, 128 * i:128 * i + 128], in_=pst)
            xT.append(t)

        # ---- theta [128, 256]; range-reduce via 2 chained add_range_wrap
        # (safe for |theta| < 4.5pi); Sin table gives both sin and cos ----
        PI = float(np.pi)
        thps = gpt([128, NPC])
        nc.tensor.matmul(
            out=thps, lhsT=fpat,
            rhs=posT,
            start=True, stop=True)
        cpat = sb.tile([128, NPC], f32, tag="cpat")
        spat = sb.tile([128, NPC], f32, tag="spat")
        for (tgt, shift) in ((spat, 0.0), (cpat, PI / 2)):
            w1 = sb.tile([128, NPC], f32, tag=f"w1{shift}", name="w1")
            nc.vector.add_range_wrap(out=w1, in_=thps, shift=shift,
                                     bound=PI, period=2 * PI)
            w2 = sb.tile([128, NPC], f32, tag=f"w2{shift}", name="w2")
            nc.vector.add_range_wrap(out=w2, in_=w1, shift=0.0,
                                     bound=PI, period=2 * PI)
            nc.scalar.activation(out=tgt, in_=w2, func=AF.Sin)

        # ---- projections (transposed) + RoPE + parity-interleaved spread.
        # qz[t] [128,128]: slot rows 32s+16P+j, cols (pair, src i) -- both
        # parities' q stacked per column.  kz[t] [128, (pr, P, i)]: same rows,
        # block-diag in data (parity P data only in P's 16-row sub-slot). ----
        def proj_m(w, m):
            ps = gpt([128, NPC])
            for k in range(3):
                nc.tensor.matmul(
                    out=ps,
                    lhsT=w[:, k, 128 * m:128 * m + 128],
                    rhs=xT[k],
                    start=(k == 0), stop=(k == 2))
            return ps

        qz, kz = [], []

        def copy_scalar(out, in_):
            # psum->sbuf copy on the scalar engine (idle during proj phase)
            nc.scalar.activation(out=out, in_=in_, func=AF.Copy)

        for m in range(3):
            for (w, out_tiles, isq, tg) in ((wq, qz, True, "q"), (wk, kz, False, "k")):
                qt = proj_m(w, m)
                a = sb.tile([128, 2, 2, 64], bf16, tag=f"ra{tg}{m}")
                b = sb.tile([128, 2, 2, 64], bf16, tag=f"rb{tg}{m}")
                nc.vector.tensor_mul(out=a, in0=qt, in1=cpat)
                nc.vector.tensor_mul(out=b, in0=qt, in1=spat)
                for half in range(2):
                    hs = slice(64 * half, 64 * half + 64)
                    if isq:
                        sp = spt([128, 128])
                        for P in range(2):
                            nc.tensor.matmul(
                                out=sp, lhsT=espx[hs, 2 * P, :],
                                rhs=a[hs, :, P, :],
                                start=(P == 0), stop=False)
                            nc.tensor.matmul(
                                out=sp, lhsT=espx[hs, 2 * P + 1, :],
                                rhs=b[hs, :, P, :],
                                start=False, stop=(P == 1))
                        t = sb.tile([128, 128], bf16, tag=f"sps{tg}{2 * m + half}")
                    else:
                        sp = spt([128, 2, 2, 64])
                        for P in range(2):
                            nc.tensor.matmul(
                                out=sp[:, :, P, :], lhsT=espx[hs, 2 * P, :],
                                rhs=a[hs, :, P, :],
                                start=True, stop=False)
                            nc.tensor.matmul(
                                out=sp[:, :, P, :], lhsT=espx[hs, 2 * P + 1, :],
                                rhs=b[hs, :, P, :],
                                start=False, stop=True)
                        t = sb.tile([128, 2, 2, 64], bf16,
                                    tag=f"sps{tg}{2 * m + half}")
                    nc.vector.tensor_copy(out=t, in_=sp)
                    out_tiles.append(t)

        # ---- V_aug [256, 408] untransposed (+ ones cols via K=1 matmul) ----
        vau = []
        for i in range(2):
            ps = gpt([128, CAUG])
            for k in range(3):
                nc.tensor.matmul(
                    out=ps,
                    lhsT=xT[k][:, 128 * i:128 * i + 128],
                    rhs=wvil[:, k, :],
                    start=(k == 0), stop=False)
            nc.tensor.matmul(
                out=ps, lhsT=onesrow, rhs=vseed,
                start=False, stop=True)
            t = sb.tile([128, CAUG], bf16, tag=f"vau{i}")
            copy_scalar(t, ps)
            vau.append(t)

        # ---- scores S^T + exp, per graph-pair: ONE block-diag matmul per
        # (gh, pair) covers both parities.  head gh -> bank gh%4 (512-col
        # block), col 64*(gh//4); rows 64*P via the kz data layout. ----
        def scol(gh):
            return 512 * (gh % 4) + 64 * (gh // 4)

        expst = []
        for pair in range(2):
            stp = ps_att.tile([128, 4 * 512], f32, tag="stps")
            et = sb.tile([128, 4 * 512], bf16, tag=f"expst{pair}")
            # 2 chunks per bank; chunk 0 (tiles 0-2 cols) is emitted as
            # soon as its scores are done so AV quads 0-2 start early
            for gh in range(GH):
                tilei, slot = divmod(gh, 4)
                lo = 32 * slot
                nc.tensor.matmul(
                    out=stp[:, scol(gh):scol(gh) + 64],
                    lhsT=kz[tilei][lo:lo + 32, pair, :, :],
                    rhs=qz[tilei][lo:lo + 32, 64 * pair:64 * pair + 64],
                    start=True, stop=True,
                    tile_position=(lo, 0))
                if gh == 11:
                    for b in range(4):
                        co = 512 * b
                        nc.scalar.activation(
                            out=et[:, co:co + 192], in_=stp[:, co:co + 192],
                            func=AF.Exp, scale=0.25)
            for b in range(4):
                co = 512 * b + 192
                nc.scalar.activation(
                    out=et[:, co:co + 192], in_=stp[:, co:co + 192],
                    func=AF.Exp, scale=0.25)
            expst.append(et)

        # ---- AV (+den row) per (pair, quad): two [128,64] psum tiles so
        # concurrent graph parities use distinct banks. avsb columns land in
        # natural node order: col = 256*qd + 128*pair + 64*g01 + i64.
        # (no memsets: psum slots hold finite data from the spread phase,
        # and pad rows are zero-masked by rdsp/wo_sp downstream) ----
        avsb = sb.tile([128, 6 * 256], bf16, tag="avsb")
        for pair in range(2):
            for qd in range(6):
                mk = spt if (qd % 2 == 0) else (lambda sh: gpt(sh))
                avt = [mk([128, 64]) for _ in range(2)]
                for a in range(4):
                    gh = 4 * qd + a
                    for g01 in range(2):
                        nc.tensor.matmul(
                            out=avt[g01][32 * a:32 * a + VW, :],
                            lhsT=vau[pair][64 * g01:64 * g01 + 64,
                                           VW * gh:VW * gh + VW],
                            rhs=expst[pair][64 * g01:64 * g01 + 64,
                                            scol(gh):scol(gh) + 64],
                            start=True, stop=True,
                            tile_position=(64 * g01, 32 * a))
                cbase = 256 * qd + 128 * pair
                for g01 in range(2):
                    nc.vector.tensor_copy(
                        out=avsb[:, cbase + 64 * g01:cbase + 64 * g01 + 64],
                        in_=avt[g01])

        # ---- merged tail: den extract (both pairs per quad), reciprocal,
        # broadcast, normalize, project.  den rows (32a+16) -> denps rows
        # 4qd+a (full [24,256] write with 4 nonzero rows, accumulated) ----
        denps = gpt([GH, 2 * 128])
        for qd in range(6):
            nc.tensor.matmul(
                out=denps,
                lhsT=esel[:, GH * qd:GH * qd + GH],
                rhs=avsb[:, 256 * qd:256 * qd + 256],
                start=(qd == 0), stop=(qd == 5))
        rden = sb.tile([GH, 2 * 128], f32, tag="rden")
        nc.vector.reciprocal_approx_fast(out=rden, in_=denps)
        rdenb = sb.tile([GH, 2 * 128], bf16, tag="rdenb")
        nc.vector.tensor_copy(out=rdenb, in_=rden)
        yps = [gpt([128, C]) for _ in range(2)]
        for qd in range(6):
            rdsp = spt([128, 2 * 128])
            nc.tensor.matmul(
                out=rdsp, lhsT=bsel[:, 128 * qd:128 * qd + 128],
                rhs=rdenb, start=True, stop=True)
            ot = sb.tile([128, 2 * 128], bf16, tag=f"onrm{qd}", name="onrm")
            nc.vector.tensor_mul(
                out=ot, in0=avsb[:, 256 * qd:256 * qd + 256], in1=rdsp)
            for pair in range(2):
                nc.tensor.matmul(
                    out=yps[pair], lhsT=ot[:, 128 * pair:128 * pair + 128],
                    rhs=wo_sp[:, C * qd:C * qd + C],
                    start=(qd == 0), stop=(qd == 5))
        for pair in range(2):
            yt = sb.tile([128, C], f32, tag=f"ysb{pair}", name="ysb")
            nc.vector.tensor_copy(out=yt, in_=yps[pair])
            (nc.sync if pair == 0 else nc.scalar).dma_start(
                out=y_d[128 * pair:128 * pair + 128, :], in_=yt)

    nc.compile()
    return nc


def _get_nc():
    if "nc" not in _CACHE:
        _CACHE["nc"] = _build_nc()
    return _CACHE["nc"]


def make_in_maps(inputs):
    x = np.asarray(inputs["x"], np.float32)
    pos = np.asarray(inputs["pos"], np.float32)
    prep = _host_prep(np.asarray(inputs["Wq"], np.float32),
                      np.asarray(inputs["Wk"], np.float32),
                      np.asarray(inputs["Wv"], np.float32),
                      np.asarray(inputs["Wo"], np.float32),
                      np.asarray(inputs["rope_freqs"], np.float32))
    in_maps = []
    for c in range(NCORES):
        sl = slice(c * NPC, (c + 1) * NPC)
        m = dict(prep)
        xs = x[sl]
        m["x"] = np.ascontiguousarray(
            xs.reshape(2, 128, C).transpose(1, 0, 2).reshape(128, 2 * C)
            .astype(BF))
        m["posT"] = np.ascontiguousarray(pos[sl].T.astype(np.float32))
        in_maps.append(m)
    return in_maps


def kernel(**inputs):
    from concourse.bass_utils import run_bass_kernel_spmd

    in_maps = make_in_maps(inputs)

    nc = _get_nc()
    res = run_bass_kernel_spmd(nc, in_maps, core_ids=list(range(NCORES)))
    out = np.concatenate([res.results[c]["y"] for c in range(NCORES)], axis=0)
    return out.astype(np.float32)
